# revision 12
# baseline (speedup 1.0000x reference)
"""Bass/Trainium2 kernel for nn_MultiHeadAttention (T5-style rel-bias causal MHA).

Sharding: 8 cores = 2 batches x 4 head-groups (4 heads of 64 dims each).
Each core: projects q/k/v for its 256 proj rows (f16 operands, f32 PSUM),
runs causal attention with the T5 relative bias applied multiplicatively
after exp (near-diagonal blocks multiply by a small Toeplitz exp-bias
table; far blocks fold the bucket-31 constant into the exp bias), and
computes a partial out-projection. Host sums the 4 partials per batch.

The exp-bias table exploits that bucket(rel) saturates at 31 for
|rel| >= 113: only blocks with diagonal offset d0 in {-128,0,128,256,384}
see a varying bias, and all of them read windows of one [128, 1024]
Toeplitz table T_h[p, y] = exp(bias(p - y + 384)) per head. Blocks with
d0 > 0 are mostly above the causal diagonal; their scores/exp/AV are
trimmed to the valid column range [d0, 512).

Attention runs in 512-query sections (4 per core); each section's
out-projection is emitted as soon as its normalization is ready, so all
but the last section's out-proj overlaps later attention sections.
"""
import math
import sys

sys.path.insert(0, "/opt/trn_rl_repo")

import numpy as np
import ml_dtypes

from concourse import bacc
import concourse.mybir as mybir
import concourse.tile as tile
from concourse.bass_utils import run_bass_kernel_spmd

F32 = mybir.dt.float32
F32R = mybir.dt.float32r
F16 = mybir.dt.float16
BF16 = mybir.dt.bfloat16
Exp = mybir.ActivationFunctionType.Exp
MUL = mybir.AluOpType.mult

NPF16 = np.float16
NPBF16 = ml_dtypes.bfloat16

B, L, D = 2, 2048, 1024
H, HD = 16, 64
NUM_BUCKETS, MAX_DISTANCE = 32, 128
HPC = 4  # heads per core
MPC = HPC * HD  # 256 proj rows per core
N_CORES = 8
NSEC = 4  # 512-query attention sections

last_results = None  # BassKernelResults of the most recent run (for profiling)
_cached = None


def _bucket(rp: np.ndarray) -> np.ndarray:
    """T5 relative position bucket, mirrors the reference exactly."""
    sign = (rp > 0).astype(np.int32)
    n = np.abs(rp)
    max_exact = NUM_BUCKETS // 2
    n_safe = np.maximum(n, 1).astype(np.float32)
    vil = max_exact + (
        np.log(n_safe / max_exact)
        / math.log(MAX_DISTANCE / max_exact)
        * (NUM_BUCKETS - max_exact)
    ).astype(np.int32)
    vil = np.minimum(vil, NUM_BUCKETS - 1)
    buckets = np.where(n < max_exact, n, vil) + sign * max_exact
    return np.clip(buckets, 0, NUM_BUCKETS - 1)


def _build():
    nc = bacc.Bacc(trn_type="TRN2")

    qT_in = nc.dram_tensor("qT_in", [D, L], F16, kind="ExternalInput")
    kT_in = nc.dram_tensor("kT_in", [D, L], F16, kind="ExternalInput")
    vT_in = nc.dram_tensor("vT_in", [D, L], F16, kind="ExternalInput")
    wq_in = nc.dram_tensor("wq_in", [128, 8, MPC], F16, kind="ExternalInput")
    wk_in = nc.dram_tensor("wk_in", [128, 8, MPC], F16, kind="ExternalInput")
    wv_in = nc.dram_tensor("wv_in", [128, 8, MPC], F16, kind="ExternalInput")
    wo_in = nc.dram_tensor("wo_in", [128, 2, D], F16, kind="ExternalInput")
    bq_in = nc.dram_tensor("bq_in", [128, 2], F32, kind="ExternalInput")
    bk_in = nc.dram_tensor("bk_in", [128, 2], F32, kind="ExternalInput")
    eb_in = nc.dram_tensor("eb_in", [HPC, 128, 1024], BF16, kind="ExternalInput")
    c31_in = nc.dram_tensor("c31_in", [128, HPC], F32, kind="ExternalInput")
    outT = nc.dram_tensor("outT", [D, L], F32, kind="ExternalOutput")

    with tile.TileContext(nc) as tc:
        with (
            tc.tile_pool(name="res", bufs=1) as pr,
            tc.tile_pool(name="qkv", bufs=1) as pqkv,
        ):
            bq = pr.tile([128, 2], F32)
            bk = pr.tile([128, 2], F32)
            c31 = pr.tile([128, HPC], F32)
            # warm the ACT exp table early, off the critical path
            warm = pr.tile([1, 2], F32)
            nc.vector.memset(warm[:], 0.0)
            nc.scalar.activation(warm[:], warm[:], Exp)
            ones_v = pr.tile([1, HD], F32R)
            nc.vector.memset(ones_v[:].bitcast(F32), 1.0)

            wo = pr.tile([128, 2, D], F16)
            ebandt = [pr.tile([128, 1024], BF16, name=f"eb{h}") for h in range(HPC)]

            qTz = []
            for hh in range(HPC):
                t = pqkv.tile([128, L], F16, name=f"qtz{hh}")
                nc.vector.memset(t[:].bitcast(F32), 0.0)
                qTz.append(t)
            kTt = [pqkv.tile([128, L], F16, name=f"kt{mm}") for mm in range(2)]
            vxg = []
            for g in range(4):
                t = pqkv.tile([128, 4, HPC, HD + 1], BF16, name=f"vx{g}")
                nc.vector.memset(t[:, :, :, HD], 1.0)
                vxg.append(t)
            y_norm_qs = [
                pqkv.tile([128, 2, 1024], F16, name=f"yn{qq}")
                for qq in range(2)
            ]

            # ---------------- projections ----------------
            with (
                tc.tile_pool(name="proj", bufs=1) as pp,
                tc.tile_pool(name="stg", bufs=8) as pstg,
                tc.tile_pool(name="stgv", bufs=8) as pstgv,
                tc.tile_pool(name="ppsum", bufs=8, space="PSUM") as pps,
            ):
                dma_engs = [nc.sync, nc.scalar]
                wq = pp.tile([128, 8, MPC], F16)
                # split so the first matmul's weight chunk lands first
                nc.sync.dma_start(wq[:, 0:1, :], wq_in[:, 0:1, :])
                nc.sync.dma_start(wq[:, 1:8, :], wq_in[:, 1:8, :])
                wk = pp.tile([128, 8, MPC], F16)
                wv = pp.tile([128, 8, MPC], F16)
                nc.sync.dma_start(bq[:], bq_in[:])
                nc.scalar.dma_start(bk[:], bk_in[:])
                nc.scalar.dma_start(c31[:], c31_in[:])

                # q/k: transposed locals [m, l] = W_c @ x.T (+bias)
                for src_d, w_sb, b_sb, dst in (
                    (qT_in, wq, bq, None),
                    (kT_in, wk, bk, kTt),
                ):
                    psums = [
                        pps.tile([128, 512], F32, tag="qk", name=f"qkp{i}")
                        for i in range(8)
                    ]
                    for kc in range(8):
                        stg = pstg.tile([128, L], F16, tag="stage")
                        off = 1 if dst is None else 0
                        eng = dma_engs[(kc + off) % len(dma_engs)]
                        if dst is None and kc == 0:
                            # fine-grained first chunk: matmul n can start
                            # as soon as its quarter arrives
                            for n in range(4):
                                eng.dma_start(
                                    stg[:, 512 * n : 512 * n + 512],
                                    src_d[0:128, 512 * n : 512 * n + 512],
                                )
                        else:
                            eng.dma_start(
                                stg[:], src_d[128 * kc : 128 * kc + 128, :]
                            )
                        for m in range(2):
                            for n in range(4):
                                nc.tensor.matmul(
                                    psums[m * 4 + n][:],
                                    w_sb[:, kc, 128 * m : 128 * m + 128],
                                    stg[:, 512 * n : 512 * n + 512],
                                    start=(kc == 0),
                                    stop=(kc == 7),
                                )
                        if dst is None and kc == 3:
                            # k weights arrive while q matmuls still run
                            nc.scalar.dma_start(wk[:], wk_in[:])
                    for m in range(2):
                        for n in range(4):
                            if dst is None:
                                for sub in range(2):
                                    pb = 64 * sub
                                    nc.vector.tensor_scalar_add(
                                        qTz[2 * m + sub][
                                            pb : pb + 64,
                                            512 * n : 512 * n + 512,
                                        ],
                                        psums[m * 4 + n][pb : pb + 64, :],
                                        b_sb[pb : pb + 64, m : m + 1],
                                    )
                            else:
                                nc.vector.tensor_scalar_add(
                                    kTt[m][:, 512 * n : 512 * n + 512],
                                    psums[m * 4 + n][:],
                                    b_sb[:, m : m + 1],
                                )

                # v: natural layout [l, m]; lhsT = staged vT chunks
                nc.scalar.dma_start(wv[:], wv_in[:])
                stgv = []
                for kc in range(8):
                    s = pstgv.tile([128, L], F16, tag="vstage")
                    eng = dma_engs[kc % len(dma_engs)]
                    eng.dma_start(s[:], vT_in[128 * kc : 128 * kc + 128, :])
                    stgv.append(s)
                # small late tables: out-proj weights + exp-bias bands
                nc.sync.dma_start(wo[:], wo_in[:])
                for h in range(HPC):
                    dma_engs[h % 2].dma_start(ebandt[h][:], eb_in[h])
                for grp in range(2):
                    psv = [
                        pps.tile([128, MPC], F32, tag="qk", name=f"vps{i}")
                        for i in range(8)
                    ]
                    for kc in range(8):
                        for i in range(8):
                            li = grp * 8 + i
                            nc.tensor.matmul(
                                psv[i][:],
                                stgv[kc][:, 128 * li : 128 * li + 128],
                                wv[:, kc, :],
                                start=(kc == 0),
                                stop=(kc == 7),
                            )
                    for i in range(8):
                        li = grp * 8 + i
                        nc.vector.tensor_copy(
                            vxg[li // 4][:, li % 4, :, 0:HD],
                            psv[i][:].rearrange("p (h d) -> p h d", h=HPC),
                        )

            # ---------------- attention + overlapped out-projection ------
            # Scores use full K=128 contraction: lhsT carries BOTH heads of
            # the m-tile; the zero rows of qT_z kill the other head exactly.
            # K=128 keeps the PE activity monitor warm (K=64 never warms).
            with (
                tc.tile_pool(name="es", bufs=8) as pes,
                tc.tile_pool(name="misc", bufs=2) as pmisc,
                tc.tile_pool(name="ost", bufs=8) as post,
                tc.tile_pool(name="spsum", bufs=3, space="PSUM") as psc,
                tc.tile_pool(name="ypsum", bufs=2, space="PSUM") as psy,
                tc.tile_pool(name="rpsum", bufs=1, space="PSUM") as psr,
                tc.tile_pool(name="opsum", bufs=2, space="PSUM") as pso,
            ):
                def _emit_norm(item):
                    rrow, pb, mt, qsec = item
                    prep = psr.tile([HD, 512], F32, tag="rep", name="prep")
                    nc.tensor.matmul(
                        prep[:], ones_v[:], rrow[:], start=True, stop=True
                    )
                    prep_sb = pmisc.tile([128, 512], F32, tag="prep")
                    nc.vector.tensor_copy(prep_sb[pb : pb + 64, :], prep[:])
                    qsi, qoff = qsec // 2, 512 * (qsec % 2)
                    ysl = y_norm_qs[qsi][
                        pb : pb + 64, mt, qoff : qoff + 512
                    ]
                    nc.vector.tensor_tensor(
                        ysl, ysl, prep_sb[pb : pb + 64, :], MUL
                    )

                def _emit_outproj_chunk(qsec, n):
                    qsi, qoff = qsec // 2, 512 * (qsec % 2)
                    po = pso.tile([128, 512], F32, tag="out")
                    for c in range(2):
                        nc.tensor.matmul(
                            po[:],
                            wo[:, c, 128 * n : 128 * n + 128],
                            y_norm_qs[qsi][:, c, qoff : qoff + 512],
                            start=(c == 0),
                            stop=(c == 1),
                        )
                    ost = post.tile([128, 512], F32, tag="ost")
                    nc.vector.tensor_copy(ost[:], po[:])
                    nc.sync.dma_start(
                        outT[
                            128 * n : 128 * n + 128,
                            512 * qsec : 512 * qsec + 512,
                        ],
                        ost[:],
                    )

                pending_norm = None
                oproj_q = []  # deferred out-proj chunks for finished qsec
                for qsec in range(NSEC):
                    q0 = 512 * qsec
                    n_live = min(4 * (qsec + 1), 16)
                    for h in range(HPC):
                        mt = h // 2
                        yT = psy.tile([HD + 1, 512], F32, tag="yT")
                        pend = []  # up to 2 deep: (ki, es, xs)
                        for ki in range(n_live):
                            d0 = 128 * ki - q0
                            xs = max(d0, 0)  # first causally-valid column
                            sp = psc.tile([128, 512], F32, tag="score")
                            nc.tensor.matmul(
                                sp[:, xs:],
                                kTt[mt][:, 128 * ki : 128 * ki + 128],
                                qTz[h][:, q0 + xs : q0 + 512],
                                start=True,
                                stop=True,
                            )
                            es = pes.tile([128, 512], BF16, tag="es")
                            if d0 <= -256:
                                nc.scalar.activation(
                                    es[:], sp[:], Exp,
                                    bias=c31[:, h : h + 1],
                                )
                            else:
                                nc.scalar.activation(
                                    es[:, xs:], sp[:, xs:], Exp
                                )
                                nc.vector.tensor_tensor(
                                    es[:, xs:],
                                    es[:, xs:],
                                    ebandt[h][:, 384 - d0 + xs : 896 - d0],
                                    MUL,
                                )
                            pend.append((ki, es, xs))
                            if len(pend) > 2 or (
                                len(pend) > 1 and ki == n_live - 1
                            ):
                                pki, pes_t, pxs = pend.pop(0)
                                nc.tensor.matmul(
                                    yT[:, pxs:],
                                    vxg[pki // 4][:, pki % 4, h, :],
                                    pes_t[:, pxs:],
                                    start=(pki == 0),
                                    stop=(pki == n_live - 1),
                                )
                            # interleave a deferred out-proj chunk; hold a
                            # few back in the last sections to fill the
                            # final recip-chain PE bubble
                            cad = 16 if qsec == NSEC - 1 else 4
                            if (
                                oproj_q
                                and ki % cad == 2
                                and not (qsec == NSEC - 1 and h == HPC - 1)
                            ):
                                _emit_outproj_chunk(*oproj_q.pop(0))
                        for pki, pes_t, pxs in pend:
                            nc.tensor.matmul(
                                yT[:, pxs:],
                                vxg[pki // 4][:, pki % 4, h, :],
                                pes_t[:, pxs:],
                                start=(pki == 0),
                                stop=(pki == n_live - 1),
                            )
                        # evacuate yT (unnormalized) into its y_norm slot
                        # and kick off the reciprocal chain; the DMA round
                        # trip to [128, 4] keeps the DVE reciprocal cost at
                        # free-size 4. The replication + in-place multiply
                        # for the PREVIOUS section is emitted now, so the
                        # PE never stalls on the recip chain.
                        pb = 64 * (h % 2)
                        qsi, qoff = qsec // 2, 512 * (qsec % 2)
                        nc.vector.tensor_copy(
                            y_norm_qs[qsi][
                                pb : pb + 64, mt, qoff : qoff + 512
                            ],
                            yT[0:HD, :],
                        )
                        dcp = pmisc.tile([1, 512], F32, tag="dcp")
                        nc.vector.tensor_copy(dcp[:], yT[HD : HD + 1, :])
                        dT = pmisc.tile([128, 4], F32, tag="dT")
                        nc.scalar.dma_start(dT[:], dcp[:])
                        rT = pmisc.tile([128, 4], F32R, tag="rT")
                        with nc.allow_low_precision(
                            reason="softmax recip f32r"
                        ):
                            nc.vector.reciprocal(rT[:], dT[:])
                        rrow = pmisc.tile([1, 512], F32R, tag="rrow")
                        nc.scalar.dma_start(rrow[:], rT[:])
                        if pending_norm is not None:
                            _emit_norm(pending_norm)
                            if pending_norm[1:3] == (64, 1):
                                # that was (qsec', h=3): queue its out-proj
                                oproj_q.extend(
                                    (pending_norm[3], n) for n in range(8)
                                )
                        pending_norm = (rrow, pb, mt, qsec)
                # held-back chunks first: they fill the PE bubble while the
                # final section's recip chain is in flight
                while oproj_q:
                    _emit_outproj_chunk(*oproj_q.pop(0))
                _emit_norm(pending_norm)
                for n in range(8):
                    _emit_outproj_chunk(NSEC - 1, n)

    nc.finalize()
    return nc


def _host_tables(rel_emb: np.ndarray):
    """Per-head exp-bias Toeplitz tables T_h[p, y] = f_h(p - y + 384)."""
    p = np.arange(128)[:, None]
    y = np.arange(1024)[None, :]
    rp = p - y + 384  # relative position key - query
    buckets = _bucket(rp)
    bands = []
    c31s = []
    for h in range(H):
        vals = np.exp(rel_emb[buckets, h].astype(np.float64))
        vals = np.where(rp > 0, 0.0, vals)  # causal mask
        bands.append(vals.astype(NPBF16))
        c31s.append(np.float32(rel_emb[31, h]))
    return bands, c31s


def _numpy_ref(query, key, value, attn_mask, key_padding_mask,
               Wq, bq, Wk, bk, Wv, bv, Wo, bo, rel_emb):
    """Exact numpy fallback for unexpected mask patterns."""
    q = (query @ Wq.T + bq).reshape(B, L, H, HD).transpose(0, 2, 1, 3)
    k = (key @ Wk.T + bk).reshape(B, L, H, HD).transpose(0, 2, 1, 3)
    v = (value @ Wv.T + bv).reshape(B, L, H, HD).transpose(0, 2, 1, 3)
    scores = np.einsum("bhqd,bhkd->bhqk", q, k) / math.sqrt(HD)
    rp = np.arange(L, dtype=np.int64)[None, :] - np.arange(L, dtype=np.int64)[:, None]
    rel = rel_emb[_bucket(rp)].transpose(2, 0, 1)
    scores = scores + rel[None]
    scores = np.where(attn_mask[None, None], scores, -np.inf)
    scores = np.where(key_padding_mask[:, None, None, :], scores, -np.inf)
    scores = scores - scores.max(-1, keepdims=True)
    e = np.exp(scores)
    attn = e / e.sum(-1, keepdims=True)
    out = np.einsum("bhqk,bhkd->bhqd", attn, v)
    out = out.transpose(0, 2, 1, 3).reshape(B, L, D)
    return (out @ Wo.T + bo).astype(np.float32)


def kernel(**inputs) -> np.ndarray:
    global _cached, last_results
    inp = {k: np.asarray(v) for k, v in inputs.items()}
    query, key, value = inp["query"], inp["key"], inp["value"]
    attn_mask, kpm = inp["attn_mask"], inp["key_padding_mask"]
    Wq, bq, Wk, bk = inp["Wq"], inp["bq"], inp["Wk"], inp["bk"]
    Wv, bv, Wo, bo = inp["Wv"], inp["bv"], inp["Wo"], inp["bo"]
    rel_emb = inp["rel_emb"]

    causal = np.array_equal(attn_mask, np.tril(np.ones((L, L), bool)))
    if not (causal and kpm.all()):
        return _numpy_ref(**inp)

    if _cached is None:
        _cached = _build()
    nc = _cached

    bands, c31s = _host_tables(rel_emb)

    def _rearr_w(w_slice):  # [MPC, D] row-major weights -> [128, 8, MPC]
        arr = np.ascontiguousarray(w_slice.T)  # [D, MPC]
        return arr.reshape(8, 128, MPC).transpose(1, 0, 2).astype(NPF16)

    in_maps = []
    for c in range(N_CORES):
        b, hg = c // HPC, c % HPC
        rows = slice(MPC * hg, MPC * hg + MPC)
        heads = range(HPC * hg, HPC * hg + HPC)
        wo_c = np.ascontiguousarray(Wo[:, rows].T)  # [MPC, D]
        in_maps.append({
            "qT_in": query[b].T.astype(NPF16),
            "kT_in": key[b].T.astype(NPF16),
            "vT_in": value[b].T.astype(NPF16),
            "wq_in": _rearr_w(Wq[rows] / math.sqrt(HD)),
            "wk_in": _rearr_w(Wk[rows]),
            "wv_in": _rearr_w(Wv[rows]),
            "wo_in": wo_c.reshape(2, 128, D).transpose(1, 0, 2).astype(NPF16),
            "bq_in": np.ascontiguousarray(
                (bq[rows] / math.sqrt(HD)).reshape(2, 128).T.astype(np.float32)
            ),
            "bk_in": np.ascontiguousarray(
                bk[rows].reshape(2, 128).T.astype(np.float32)
            ),
            "eb_in": np.stack([bands[h] for h in heads]),
            "c31_in": np.tile(
                np.array([c31s[h] for h in heads], np.float32), (128, 1)
            ),
        })

    res = run_bass_kernel_spmd(nc, in_maps, list(range(N_CORES)))
    last_results = res

    bo_eff = (
        bo.astype(np.float64) + bv.astype(np.float64) @ Wo.T.astype(np.float64)
    )
    out = np.empty((B, L, D), np.float32)
    for b in range(B):
        acc = np.zeros((D, L), np.float64)
        for hg in range(HPC):
            acc += res.results[b * HPC + hg]["outT"]
        out[b] = (acc.T + bo_eff[None, :]).astype(np.float32)
    return out


# revision 13
# speedup vs baseline: 1.0630x; 1.0630x over previous
"""Bass/Trainium2 kernel for nn_MultiHeadAttention (T5-style rel-bias causal MHA).

Sharding: 8 cores = 2 batches x 4 head-groups (4 heads of 64 dims each).
Each core: projects q/k/v for its 256 proj rows (f16 operands, f32 PSUM),
runs causal attention with the T5 relative bias applied multiplicatively
after exp (near-diagonal blocks multiply by a small Toeplitz exp-bias
table; far blocks fold the bucket-31 constant into the exp bias), and
computes a partial out-projection. Host sums the 4 partials per batch.

The exp-bias table exploits that bucket(rel) saturates at 31 for
|rel| >= 113: only blocks with diagonal offset d0 in {-128,0,128,256,384}
see a varying bias, and all of them read windows of one [128, 1024]
Toeplitz table T_h[p, y] = exp(bias(p - y + 384)) per head. Blocks with
d0 > 0 are mostly above the causal diagonal; their scores/exp/AV are
trimmed to the valid column range [d0, 512).

Attention runs in 512-query sections (4 per core); each section's
out-projection is emitted as soon as its normalization is ready, so all
but the last section's out-proj overlaps later attention sections.
"""
import math
import sys

sys.path.insert(0, "/opt/trn_rl_repo")

import numpy as np
import ml_dtypes

from concourse import bacc
import concourse.mybir as mybir
import concourse.tile as tile
from concourse.bass_utils import run_bass_kernel_spmd

F32 = mybir.dt.float32
F32R = mybir.dt.float32r
F16 = mybir.dt.float16
BF16 = mybir.dt.bfloat16
Exp = mybir.ActivationFunctionType.Exp
MUL = mybir.AluOpType.mult

NPF16 = np.float16
NPBF16 = ml_dtypes.bfloat16

B, L, D = 2, 2048, 1024
H, HD = 16, 64
NUM_BUCKETS, MAX_DISTANCE = 32, 128
HPC = 4  # heads per core
MPC = HPC * HD  # 256 proj rows per core
N_CORES = 8
NSEC = 4  # 512-query attention sections

last_results = None  # BassKernelResults of the most recent run (for profiling)
_cached = None


def _bucket(rp: np.ndarray) -> np.ndarray:
    """T5 relative position bucket, mirrors the reference exactly."""
    sign = (rp > 0).astype(np.int32)
    n = np.abs(rp)
    max_exact = NUM_BUCKETS // 2
    n_safe = np.maximum(n, 1).astype(np.float32)
    vil = max_exact + (
        np.log(n_safe / max_exact)
        / math.log(MAX_DISTANCE / max_exact)
        * (NUM_BUCKETS - max_exact)
    ).astype(np.int32)
    vil = np.minimum(vil, NUM_BUCKETS - 1)
    buckets = np.where(n < max_exact, n, vil) + sign * max_exact
    return np.clip(buckets, 0, NUM_BUCKETS - 1)


def _build():
    nc = bacc.Bacc(trn_type="TRN2")

    qT_in = nc.dram_tensor("qT_in", [D, L], F16, kind="ExternalInput")
    kT_in = nc.dram_tensor("kT_in", [D, L], F16, kind="ExternalInput")
    vT_in = nc.dram_tensor("vT_in", [D, L], F16, kind="ExternalInput")
    wq_in = nc.dram_tensor("wq_in", [128, 8, MPC], F16, kind="ExternalInput")
    wk_in = nc.dram_tensor("wk_in", [128, 8, MPC], F16, kind="ExternalInput")
    wv_in = nc.dram_tensor("wv_in", [128, 8, MPC], F16, kind="ExternalInput")
    wo_in = nc.dram_tensor("wo_in", [128, 2, D], F16, kind="ExternalInput")
    bq_in = nc.dram_tensor("bq_in", [128, 2], F32, kind="ExternalInput")
    bk_in = nc.dram_tensor("bk_in", [128, 2], F32, kind="ExternalInput")
    eb_in = nc.dram_tensor("eb_in", [HPC, 128, 1024], BF16, kind="ExternalInput")
    c31_in = nc.dram_tensor("c31_in", [128, HPC], F32, kind="ExternalInput")
    outT = nc.dram_tensor("outT", [D, L], F32, kind="ExternalOutput")

    with tile.TileContext(nc) as tc:
        with (
            tc.tile_pool(name="res", bufs=1) as pr,
            tc.tile_pool(name="qkv", bufs=1) as pqkv,
        ):
            bq = pr.tile([128, 2], F32)
            bk = pr.tile([128, 2], F32)
            c31 = pr.tile([128, HPC], F32)
            # warm the ACT exp table early, off the critical path
            warm = pr.tile([1, 2], F32)
            nc.vector.memset(warm[:], 0.0)
            nc.scalar.activation(warm[:], warm[:], Exp)
            ones_v = pr.tile([1, HD], F32R)
            nc.vector.memset(ones_v[:].bitcast(F32), 1.0)

            wo = pr.tile([128, 2, D], F16)
            ebandt = [pr.tile([128, 1024], BF16, name=f"eb{h}") for h in range(HPC)]

            qTz = []
            for hh in range(HPC):
                t = pqkv.tile([128, L], F16, name=f"qtz{hh}")
                nc.vector.memset(t[:].bitcast(F32), 0.0)
                qTz.append(t)
            kTt = [pqkv.tile([128, L], F16, name=f"kt{mm}") for mm in range(2)]
            vxg = []
            for g in range(4):
                t = pqkv.tile([128, 4, HPC, HD + 1], BF16, name=f"vx{g}")
                nc.vector.memset(t[:, :, :, HD], 1.0)
                vxg.append(t)
            y_norm_qs = [
                pqkv.tile([128, 2, 1024], F16, name=f"yn{qq}")
                for qq in range(2)
            ]

            # ---------------- projections ----------------
            with (
                tc.tile_pool(name="proj", bufs=1) as pp,
                tc.tile_pool(name="stg", bufs=8) as pstg,
                tc.tile_pool(name="stgv", bufs=8) as pstgv,
                tc.tile_pool(name="ppsum", bufs=8, space="PSUM") as pps,
            ):
                dma_engs = [nc.sync, nc.scalar]
                wq = pp.tile([128, 8, MPC], F16)
                # split so the first matmul's weight chunk lands first
                nc.sync.dma_start(wq[:, 0:1, :], wq_in[:, 0:1, :])
                nc.sync.dma_start(wq[:, 1:8, :], wq_in[:, 1:8, :])
                wk = pp.tile([128, 8, MPC], F16)
                wv = pp.tile([128, 8, MPC], F16)
                nc.sync.dma_start(bq[:], bq_in[:])
                nc.scalar.dma_start(bk[:], bk_in[:])
                nc.scalar.dma_start(c31[:], c31_in[:])

                # q/k: transposed locals [m, l] = W_c @ x.T (+bias)
                for src_d, w_sb, b_sb, dst in (
                    (qT_in, wq, bq, None),
                    (kT_in, wk, bk, kTt),
                ):
                    psums = [
                        pps.tile([128, 512], F32, tag="qk", name=f"qkp{i}")
                        for i in range(8)
                    ]
                    for kc in range(8):
                        stg = pstg.tile([128, L], F16, tag="stage")
                        off = 1 if dst is None else 0
                        eng = dma_engs[(kc + off) % len(dma_engs)]
                        if dst is None and kc == 0:
                            # fine-grained first chunk: matmul n can start
                            # as soon as its quarter arrives
                            for n in range(4):
                                eng.dma_start(
                                    stg[:, 512 * n : 512 * n + 512],
                                    src_d[0:128, 512 * n : 512 * n + 512],
                                )
                        else:
                            eng.dma_start(
                                stg[:], src_d[128 * kc : 128 * kc + 128, :]
                            )
                        for m in range(2):
                            for n in range(4):
                                nc.tensor.matmul(
                                    psums[m * 4 + n][:],
                                    w_sb[:, kc, 128 * m : 128 * m + 128],
                                    stg[:, 512 * n : 512 * n + 512],
                                    start=(kc == 0),
                                    stop=(kc == 7),
                                )
                        if dst is None and kc == 3:
                            # k weights arrive while q matmuls still run
                            nc.scalar.dma_start(wk[:], wk_in[:])
                    for m in range(2):
                        for n in range(4):
                            if dst is None:
                                for sub in range(2):
                                    pb = 64 * sub
                                    nc.vector.tensor_scalar_add(
                                        qTz[2 * m + sub][
                                            pb : pb + 64,
                                            512 * n : 512 * n + 512,
                                        ],
                                        psums[m * 4 + n][pb : pb + 64, :],
                                        b_sb[pb : pb + 64, m : m + 1],
                                    )
                            else:
                                nc.vector.tensor_scalar_add(
                                    kTt[m][:, 512 * n : 512 * n + 512],
                                    psums[m * 4 + n][:],
                                    b_sb[:, m : m + 1],
                                )

                # v: natural layout [l, m]; lhsT = staged vT chunks
                nc.scalar.dma_start(wv[:], wv_in[:])
                stgv = []
                for kc in range(8):
                    s = pstgv.tile([128, L], F16, tag="vstage")
                    eng = dma_engs[kc % len(dma_engs)]
                    eng.dma_start(s[:], vT_in[128 * kc : 128 * kc + 128, :])
                    stgv.append(s)
                # small late tables: out-proj weights + exp-bias bands
                nc.sync.dma_start(wo[:], wo_in[:])
                for h in range(HPC):
                    dma_engs[h % 2].dma_start(ebandt[h][:], eb_in[h])
                for grp in range(2):
                    psv = [
                        pps.tile([128, MPC], F32, tag="qk", name=f"vps{i}")
                        for i in range(8)
                    ]
                    for kc in range(8):
                        for i in range(8):
                            li = grp * 8 + i
                            nc.tensor.matmul(
                                psv[i][:],
                                stgv[kc][:, 128 * li : 128 * li + 128],
                                wv[:, kc, :],
                                start=(kc == 0),
                                stop=(kc == 7),
                            )
                    for i in range(8):
                        li = grp * 8 + i
                        nc.vector.tensor_copy(
                            vxg[li // 4][:, li % 4, :, 0:HD],
                            psv[i][:].rearrange("p (h d) -> p h d", h=HPC),
                        )

            # ---------------- attention + overlapped out-projection ------
            # Scores use full K=128 contraction: lhsT carries BOTH heads of
            # the m-tile; the zero rows of qT_z kill the other head exactly.
            # K=128 keeps the PE activity monitor warm (K=64 never warms).
            with (
                tc.tile_pool(name="es", bufs=8) as pes,
                tc.tile_pool(name="misc", bufs=2) as pmisc,
                tc.tile_pool(name="ost", bufs=8) as post,
                tc.tile_pool(name="spsum", bufs=3, space="PSUM") as psc,
                tc.tile_pool(name="ypsum", bufs=2, space="PSUM") as psy,
                tc.tile_pool(name="rpsum", bufs=1, space="PSUM") as psr,
                tc.tile_pool(name="opsum", bufs=2, space="PSUM") as pso,
            ):
                def _emit_norm(item):
                    rrow, pb, mt, qsec = item
                    prep = psr.tile([HD, 512], F32, tag="rep", name="prep")
                    nc.tensor.matmul(
                        prep[:], ones_v[:], rrow[:], start=True, stop=True
                    )
                    prep_sb = pmisc.tile([128, 512], F32, tag="prep")
                    nc.vector.tensor_copy(prep_sb[pb : pb + 64, :], prep[:])
                    qsi, qoff = qsec // 2, 512 * (qsec % 2)
                    ysl = y_norm_qs[qsi][
                        pb : pb + 64, mt, qoff : qoff + 512
                    ]
                    nc.vector.tensor_tensor(
                        ysl, ysl, prep_sb[pb : pb + 64, :], MUL
                    )

                def _emit_outproj_chunk(qsec, n):
                    qsi, qoff = qsec // 2, 512 * (qsec % 2)
                    po = pso.tile([128, 512], F32, tag="out")
                    for c in range(2):
                        nc.tensor.matmul(
                            po[:],
                            wo[:, c, 128 * n : 128 * n + 128],
                            y_norm_qs[qsi][:, c, qoff : qoff + 512],
                            start=(c == 0),
                            stop=(c == 1),
                        )
                    ost = post.tile([128, 512], F32, tag="ost")
                    nc.vector.tensor_copy(ost[:], po[:])
                    nc.sync.dma_start(
                        outT[
                            128 * n : 128 * n + 128,
                            512 * qsec : 512 * qsec + 512,
                        ],
                        ost[:],
                    )

                pending_norm = None
                oproj_q = []  # deferred out-proj chunks for finished qsec
                for qsec in range(NSEC):
                    q0 = 512 * qsec
                    n_live = min(4 * (qsec + 1), 16)
                    for h in range(HPC):
                        mt = h // 2
                        yT = psy.tile([HD + 1, 512], F32, tag="yT")
                        pend = []  # up to 2 deep: (ki, es, xs)
                        for ki in range(n_live):
                            d0 = 128 * ki - q0
                            xs = max(d0, 0)  # first causally-valid column
                            sp = psc.tile([128, 512], F32, tag="score")
                            nc.tensor.matmul(
                                sp[:, xs:],
                                kTt[mt][:, 128 * ki : 128 * ki + 128],
                                qTz[h][:, q0 + xs : q0 + 512],
                                start=True,
                                stop=True,
                            )
                            es = pes.tile([128, 512], BF16, tag="es")
                            if d0 <= -256:
                                nc.scalar.activation(
                                    es[:], sp[:], Exp,
                                    bias=c31[:, h : h + 1],
                                )
                            else:
                                nc.scalar.activation(
                                    es[:, xs:], sp[:, xs:], Exp
                                )
                                nc.vector.tensor_tensor(
                                    es[:, xs:],
                                    es[:, xs:],
                                    ebandt[h][:, 384 - d0 + xs : 896 - d0],
                                    MUL,
                                )
                            pend.append((ki, es, xs))
                            if len(pend) > 2 or (
                                len(pend) > 1 and ki == n_live - 1
                            ):
                                pki, pes_t, pxs = pend.pop(0)
                                nc.tensor.matmul(
                                    yT[:, pxs:],
                                    vxg[pki // 4][:, pki % 4, h, :],
                                    pes_t[:, pxs:],
                                    start=(pki == 0),
                                    stop=(pki == n_live - 1),
                                )
                            # interleave a deferred out-proj chunk; hold a
                            # few back in the last sections to fill the
                            # final recip-chain PE bubble
                            cad = 16 if qsec == NSEC - 1 else 4
                            if (
                                oproj_q
                                and ki % cad == 2
                                and not (qsec == NSEC - 1 and h == HPC - 1)
                            ):
                                _emit_outproj_chunk(*oproj_q.pop(0))
                        for pki, pes_t, pxs in pend:
                            nc.tensor.matmul(
                                yT[:, pxs:],
                                vxg[pki // 4][:, pki % 4, h, :],
                                pes_t[:, pxs:],
                                start=(pki == 0),
                                stop=(pki == n_live - 1),
                            )
                        # evacuate yT (unnormalized) into its y_norm slot
                        # and kick off the reciprocal chain; the DMA round
                        # trip to [128, 4] keeps the DVE reciprocal cost at
                        # free-size 4. The replication + in-place multiply
                        # for the PREVIOUS section is emitted now, so the
                        # PE never stalls on the recip chain.
                        pb = 64 * (h % 2)
                        qsi, qoff = qsec // 2, 512 * (qsec % 2)
                        nc.vector.tensor_copy(
                            y_norm_qs[qsi][
                                pb : pb + 64, mt, qoff : qoff + 512
                            ],
                            yT[0:HD, :],
                        )
                        dcp = pmisc.tile([1, 512], F32, tag="dcp")
                        nc.vector.tensor_copy(dcp[:], yT[HD : HD + 1, :])
                        dT = pmisc.tile([128, 4], F32, tag="dT")
                        nc.gpsimd.dma_start(dT[:], dcp[:])
                        rT = pmisc.tile([128, 4], F32R, tag="rT")
                        with nc.allow_low_precision(
                            reason="softmax recip f32r"
                        ):
                            nc.vector.reciprocal(rT[:], dT[:])
                        rrow = pmisc.tile([1, 512], F32R, tag="rrow")
                        nc.gpsimd.dma_start(rrow[:], rT[:])
                        if pending_norm is not None:
                            _emit_norm(pending_norm)
                            if pending_norm[1:3] == (64, 1):
                                # that was (qsec', h=3): queue its out-proj
                                oproj_q.extend(
                                    (pending_norm[3], n) for n in range(8)
                                )
                        pending_norm = (rrow, pb, mt, qsec)
                # held-back chunks first: they fill the PE bubble while the
                # final section's recip chain is in flight
                while oproj_q:
                    _emit_outproj_chunk(*oproj_q.pop(0))
                _emit_norm(pending_norm)
                for n in range(8):
                    _emit_outproj_chunk(NSEC - 1, n)

    nc.finalize()
    return nc


def _host_tables(rel_emb: np.ndarray):
    """Per-head exp-bias Toeplitz tables T_h[p, y] = f_h(p - y + 384)."""
    p = np.arange(128)[:, None]
    y = np.arange(1024)[None, :]
    rp = p - y + 384  # relative position key - query
    buckets = _bucket(rp)
    bands = []
    c31s = []
    for h in range(H):
        vals = np.exp(rel_emb[buckets, h].astype(np.float64))
        vals = np.where(rp > 0, 0.0, vals)  # causal mask
        bands.append(vals.astype(NPBF16))
        c31s.append(np.float32(rel_emb[31, h]))
    return bands, c31s


def _numpy_ref(query, key, value, attn_mask, key_padding_mask,
               Wq, bq, Wk, bk, Wv, bv, Wo, bo, rel_emb):
    """Exact numpy fallback for unexpected mask patterns."""
    q = (query @ Wq.T + bq).reshape(B, L, H, HD).transpose(0, 2, 1, 3)
    k = (key @ Wk.T + bk).reshape(B, L, H, HD).transpose(0, 2, 1, 3)
    v = (value @ Wv.T + bv).reshape(B, L, H, HD).transpose(0, 2, 1, 3)
    scores = np.einsum("bhqd,bhkd->bhqk", q, k) / math.sqrt(HD)
    rp = np.arange(L, dtype=np.int64)[None, :] - np.arange(L, dtype=np.int64)[:, None]
    rel = rel_emb[_bucket(rp)].transpose(2, 0, 1)
    scores = scores + rel[None]
    scores = np.where(attn_mask[None, None], scores, -np.inf)
    scores = np.where(key_padding_mask[:, None, None, :], scores, -np.inf)
    scores = scores - scores.max(-1, keepdims=True)
    e = np.exp(scores)
    attn = e / e.sum(-1, keepdims=True)
    out = np.einsum("bhqk,bhkd->bhqd", attn, v)
    out = out.transpose(0, 2, 1, 3).reshape(B, L, D)
    return (out @ Wo.T + bo).astype(np.float32)


def kernel(**inputs) -> np.ndarray:
    global _cached, last_results
    inp = {k: np.asarray(v) for k, v in inputs.items()}
    query, key, value = inp["query"], inp["key"], inp["value"]
    attn_mask, kpm = inp["attn_mask"], inp["key_padding_mask"]
    Wq, bq, Wk, bk = inp["Wq"], inp["bq"], inp["Wk"], inp["bk"]
    Wv, bv, Wo, bo = inp["Wv"], inp["bv"], inp["Wo"], inp["bo"]
    rel_emb = inp["rel_emb"]

    causal = np.array_equal(attn_mask, np.tril(np.ones((L, L), bool)))
    if not (causal and kpm.all()):
        return _numpy_ref(**inp)

    if _cached is None:
        _cached = _build()
    nc = _cached

    bands, c31s = _host_tables(rel_emb)

    def _rearr_w(w_slice):  # [MPC, D] row-major weights -> [128, 8, MPC]
        arr = np.ascontiguousarray(w_slice.T)  # [D, MPC]
        return arr.reshape(8, 128, MPC).transpose(1, 0, 2).astype(NPF16)

    in_maps = []
    for c in range(N_CORES):
        b, hg = c // HPC, c % HPC
        rows = slice(MPC * hg, MPC * hg + MPC)
        heads = range(HPC * hg, HPC * hg + HPC)
        wo_c = np.ascontiguousarray(Wo[:, rows].T)  # [MPC, D]
        in_maps.append({
            "qT_in": query[b].T.astype(NPF16),
            "kT_in": key[b].T.astype(NPF16),
            "vT_in": value[b].T.astype(NPF16),
            "wq_in": _rearr_w(Wq[rows] / math.sqrt(HD)),
            "wk_in": _rearr_w(Wk[rows]),
            "wv_in": _rearr_w(Wv[rows]),
            "wo_in": wo_c.reshape(2, 128, D).transpose(1, 0, 2).astype(NPF16),
            "bq_in": np.ascontiguousarray(
                (bq[rows] / math.sqrt(HD)).reshape(2, 128).T.astype(np.float32)
            ),
            "bk_in": np.ascontiguousarray(
                bk[rows].reshape(2, 128).T.astype(np.float32)
            ),
            "eb_in": np.stack([bands[h] for h in heads]),
            "c31_in": np.tile(
                np.array([c31s[h] for h in heads], np.float32), (128, 1)
            ),
        })

    res = run_bass_kernel_spmd(nc, in_maps, list(range(N_CORES)))
    last_results = res

    bo_eff = (
        bo.astype(np.float64) + bv.astype(np.float64) @ Wo.T.astype(np.float64)
    )
    out = np.empty((B, L, D), np.float32)
    for b in range(B):
        acc = np.zeros((D, L), np.float64)
        for hg in range(HPC):
            acc += res.results[b * HPC + hg]["outT"]
        out[b] = (acc.T + bo_eff[None, :]).astype(np.float32)
    return out
